# revision 20
# baseline (speedup 1.0000x reference)
"""NuFFT forward (cube -> visibilities) on 8 Trainium2 NeuronCores.

Algorithm (validated in numpy, rel err ~3.7e-3 absmax vs fp32 reference):
  1. F = FFT2(pad(deapodize(fftshift(cube))))  computed as two DFT matmuls
     with fftshift/deapodize/pad/ifftshift folded into host-precomputed
     complex DFT matrices (bf16 inputs, fp32 PSUM accumulation).
     Each core computes only its 134-row v-band of the 1024x1024 grid.
  2. Kaiser-Bessel interpolation via "polynomial-in-v" trick:
     wv[m,i] = sum_p A[i,p] * fv^p  (degree-7 fit, max rel err 7e-6), so
       out[c,m] = sum_j wu[m,j] sum_p fv[m]^p G_p[c, bv[m], bu[m]+j-2]
     where G_p = conv_v(F, A[:,p]) is computed as a banded matmul on PE.
     G is stored in HBM as [v, u, (c,ri,p)] so each visibility needs ONE
     contiguous 1536B gather (6 u-points x 256B), done with dma_gather.
  3. On-chip reduce (DVE): contract p with fv-powers, then j with wu.

Sharding: vis binned by bv into 8 v-bands (one per core), then into 5
v-subbands per core to keep dma_gather's int16 index space < 32767.
Overflowing buckets (pathological inputs) fall back to exact host compute.
"""

import numpy as np

import concourse.bass as bass
import concourse.bacc as bacc
import concourse.mybir as mybir
import concourse.tile as tile
from concourse import library_config
from concourse.bass_utils import run_bass_kernel_spmd

# ---------------- problem constants (hardcoded per spec) ----------------
NPIX = 512
NCHAN = 4
NVIS = 131072
GRID = 1024
J = 6
ALPHA = 2.34 * J
CELL_SIZE = 0.005
ARCSEC = np.pi / (180.0 * 3600.0)
DL = CELL_SIZE * ARCSEC
GSCALE = 1000.0 * DL * GRID      # g = klambda * GSCALE (grid units)

NCORES = 8
P = 8                            # polynomial terms per KB tap
BAND = GRID // NCORES            # 128 v-rows per core
FROWS = BAND + J                 # F band rows incl halo (134)
SUBROWS = (26, 26, 26, 26, 24)   # v-subband split (int16 index space)
ROW0 = (0, 26, 52, 78, 104)
ROWPTS = GRID + 6                # u points per stored G row (wrap dup)
PAYLOAD = 2 * NCHAN * P          # f32 per u-point: (c,ri,p) = 64 -> 256B
ELEM = 6 * PAYLOAD               # gathered elem: 384 f32 = 1536B
NSUBPAD = 3584                   # padded vis per (core, subband)
NBLK = NSUBPAD // 128            # 28
NCHUNK = 2                       # gather calls per subband
CIDX = NSUBPAD // NCHUNK         # 1792 idxs per gather call
CBLK = CIDX // 128               # 14 blocks per chunk
GBUF_F32 = BAND * ROWPTS * PAYLOAD   # per-core G buffer (f32 elems)

_F32 = mybir.dt.float32
_BF16 = mybir.dt.bfloat16
_I16 = mybir.dt.int16


# ---------------- host-side constant precompute ----------------
def _kb(d):
    t = 1.0 - (2.0 * d / J) ** 2
    with np.errstate(invalid="ignore"):
        v = np.where(t > 0, np.i0(ALPHA * np.sqrt(np.maximum(t, 0.0))) / J, 0.0)
    return v


def _kb_ft(u):
    z = np.sqrt((np.pi * J * u) ** 2 - ALPHA ** 2 + 0j)
    return np.real(np.sin(z) / z)


def _poly_coefs():
    """A[i, p]: kb(f - (i-2)) ~= sum_p A[i,p] f^p on f in [0,1)."""
    f = np.linspace(0.0, 1.0, 4001)
    V = np.vander(f, P, increasing=True)
    A = np.stack([np.linalg.lstsq(V, _kb(f - (i - 2)), rcond=None)[0]
                  for i in range(J)])
    return A  # float64 [J, P]


def _dft_matrices():
    """Effective DFT matrices absorbing fftshift, de-apodization, padding
    and ifftshift.  D[k, m] applied to raw cube index m gives spectrum
    row/col k of the oversampled grid."""
    n = (np.arange(NPIX) - NPIX // 2) / GRID
    sc = 1.0 / _kb_ft(n)                       # de-apod per img index n1
    m = np.arange(NPIX)
    n_of_m = (m + 256) % NPIX                  # img row for cube row m
    phi = (n_of_m - 256) % GRID                # padded+ifftshifted position
    k = np.arange(GRID)
    D = sc[n_of_m][None, :] * np.exp(-2j * np.pi * k[:, None] * phi[None, :] / GRID)
    return D  # complex128 [GRID, NPIX]


class _Consts:
    def __init__(self):
        A = _poly_coefs()
        self.A = A
        D = _dft_matrices()
        bf = lambda x: x.astype(np.float32).astype(np.float32)  # keep f32; cast on pack
        self.D = D
        # DuT variants [NPIX, GRID] bf16: real, imag, -imag
        DuT = D.T.copy()
        self.DuT_r = DuT.real.astype(np.float32)
        self.DuT_i = DuT.imag.astype(np.float32)
        self.DuT_ni = (-DuT.imag).astype(np.float32)
        # Per-core DvT bands [NPIX, FROWS]
        self.DvT_r = []
        self.DvT_i = []
        for c in range(NCORES):
            rows = (c * BAND - 2 + np.arange(FROWS)) % GRID
            Db = D[rows, :].T.copy()     # [NPIX, FROWS]
            self.DvT_r.append(Db.real.astype(np.float32))
            self.DvT_i.append(Db.imag.astype(np.float32))
        # Conv matrices.  G_p[v0] = sum_i A[i,p] F_local[v0+i],
        # F_local row r = global band row (c*BAND - 2 + r).
        W1 = np.zeros((P, 128, 128), np.float32)   # [p, r, v0] for r<128
        for p in range(P):
            for v0 in range(128):
                for i in range(J):
                    r = v0 + i
                    if r < 128:
                        W1[p, r, v0] = A[i, p]
        self.W1T = W1                               # lhsT layout [K=r, M=v0]
        # halo: r in [128, 134) -> v0 in [123,128): lhsT [6, P*5]
        # halo lhsT per p: [J, 32] columns = (v0 - 96), nonzero v0 >= 123;
        # accumulated into the conv PSUM rows [96,128) via a 2nd matmul
        W2 = np.zeros((P, J, 128), np.float32)
        for p in range(P):
            for v0 in range(123, 128):
                for i in range(J):
                    r = v0 + i
                    if r >= 128:
                        W2[p, r - 128, v0] = A[i, p]
        self.W2 = W2
        # A replicated for on-device wu computation, scaled by CELL^2
        self.A_wu = (A * (CELL_SIZE ** 2)).astype(np.float32)  # [J, P]


_CONSTS = None


def _consts():
    global _CONSTS
    if _CONSTS is None:
        _CONSTS = _Consts()
    return _CONSTS


def _to_bf16(x):
    return np.asarray(x, dtype=np.float32).astype(mybir.dt.np(_BF16))


# ---------------- device program ----------------
def _build_program(compile=True):
    cst = _consts()
    nc = bacc.Bacc("TRN2", target_bir_lowering=False, debug=False)

    cube_in = nc.declare_dram_parameter("cube", [NCHAN, NPIX, NPIX], _F32, isOutput=False)
    dvt_r = nc.declare_dram_parameter("dvt_r", [NPIX, FROWS], _BF16, isOutput=False)
    dvt_i = nc.declare_dram_parameter("dvt_i", [NPIX, FROWS], _BF16, isOutput=False)
    dut_r = nc.declare_dram_parameter("dut_r", [NPIX, GRID], _BF16, isOutput=False)
    dut_i = nc.declare_dram_parameter("dut_i", [NPIX, GRID], _BF16, isOutput=False)
    dut_ni = nc.declare_dram_parameter("dut_ni", [NPIX, GRID], _BF16, isOutput=False)
    w1t_in = nc.declare_dram_parameter("w1t", [P, 128, 128], _BF16, isOutput=False)
    w2_in = nc.declare_dram_parameter("w2", [P, J, 128], _BF16, isOutput=False)
    awu_in = nc.declare_dram_parameter("awu", [J * P], _F32, isOutput=False)
    idx_in = nc.declare_dram_parameter("idx", [len(SUBROWS), 128, NSUBPAD // 16], _I16, isOutput=False)
    frac_in = nc.declare_dram_parameter("frac", [2, 128, len(SUBROWS) * NBLK], _F32, isOutput=False)
    out_dram = nc.declare_dram_parameter("out", [128, len(SUBROWS) * NBLK * 8], _F32, isOutput=True)

    gbuf = nc.dram_tensor("gbuf", [GBUF_F32], _F32)

    NSUB = len(SUBROWS)

    with tile.TileContext(nc) as tc:
        with (
            tc.tile_pool(name="const", bufs=1) as cpool,
            tc.tile_pool(name="work", bufs=1) as wpool,
            tc.tile_pool(name="gather", bufs=2) as gpool,
            tc.tile_pool(name="stage", bufs=2) as spool,
        ):
            nc.gpsimd.load_library(library_config.mlp)

            # ---- constant loads ----
            cube_sb = cpool.tile([128, NCHAN, 4, NPIX], _BF16)   # [m1p, c, m1t, m2]
            for t in range(4):
                nc.gpsimd.dma_start(
                    out=cube_sb[:, :, t, :],
                    in_=cube_in[:, bass.ts(t, 128), :].transpose([1, 0, 2]),
                )
            dvt_sb = cpool.tile([128, 4, 2, FROWS], _BF16)       # [m1p, m1t, ri, k1]
            for t in range(4):
                nc.sync.dma_start(out=dvt_sb[:, t, 0, :], in_=dvt_r[bass.ts(t, 128), :])
                nc.sync.dma_start(out=dvt_sb[:, t, 1, :], in_=dvt_i[bass.ts(t, 128), :])
            dut_sb = cpool.tile([128, 4, 3, GRID], _BF16)        # [m2p, m2t, var, k2]
            for t in range(4):
                nc.sync.dma_start(out=dut_sb[:, t, 0, :], in_=dut_r[bass.ts(t, 128), :])
                nc.sync.dma_start(out=dut_sb[:, t, 1, :], in_=dut_i[bass.ts(t, 128), :])
                nc.sync.dma_start(out=dut_sb[:, t, 2, :], in_=dut_ni[bass.ts(t, 128), :])
            w1t_sb = cpool.tile([128, P, 128], _BF16)
            nc.sync.dma_start(out=w1t_sb[:], in_=w1t_in[:].transpose([1, 0, 2]))
            w2_sb = cpool.tile([J, P, 128], _BF16)
            nc.sync.dma_start(out=w2_sb[:], in_=w2_in[:].transpose([1, 0, 2]))
            awu_sb = cpool.tile([128, J * P], _F32)
            nc.sync.dma_start(out=awu_sb[:], in_=awu_in[None, :].broadcast_to([128, J * P]))
            idx_sb = cpool.tile([128, NSUB, NSUBPAD // 16], _I16)
            nc.sync.dma_start(out=idx_sb[:], in_=idx_in[:].transpose([1, 0, 2]))
            frac_sb = cpool.tile([128, 2, NSUB * NBLK], _F32)
            nc.sync.dma_start(out=frac_sb[:], in_=frac_in[:].transpose([1, 0, 2]))

            # ---- stage A: M1T[m2, k1] = (cube^T @ DvT) per (c, ri) ----
            m1t_sb = wpool.tile([128, NCHAN, 4, 2, FROWS], _BF16)  # [m2p, c, m2blk, ri, k1]
            psA = ctxA = tc.tile_pool(name="psA", bufs=4, space="PSUM")
            psA = ctxA.__enter__()
            for c in range(NCHAN):
                for blk in range(4):
                    for ri in range(2):
                        ps = psA.tile([128, FROWS], _F32, tag="psA")
                        for kt in range(4):
                            nc.tensor.matmul(
                                out=ps[:],
                                lhsT=cube_sb[:, c, kt, bass.ts(blk, 128)],
                                rhs=dvt_sb[:, kt, ri, :],
                                start=(kt == 0),
                                stop=(kt == 3),
                            )
                        nc.vector.tensor_copy(out=m1t_sb[:, c, blk, ri, :], in_=ps[:])

            ctxA.__exit__(None, None, None)

            # ---- stage B: F[k1, k2] = M1 @ Du^T  (k1 in band, 134 rows) ----
            # F_sb layout: [v-part, (k2, c, ri)] bf16; main 128 rows + 6-row tail
            f_main = wpool.tile([128, GRID, NCHAN, 2], _BF16)
            f_tail = wpool.tile([6, GRID, NCHAN, 2], _BF16)
            ctxB = tc.tile_pool(name="psB", bufs=4, space="PSUM")
            psB = ctxB.__enter__()
            for c in range(NCHAN):
                for half in range(2):
                    ks = bass.ts(half, 512)
                    ps_r = psB.tile([128, 512], _F32, tag="psB")
                    ps_i = psB.tile([128, 512], _F32, tag="psB")
                    # F_r = M1r*Dur + M1i*(-Dui);  F_i = M1r*Dui + M1i*Dur
                    for kt in range(4):
                        nc.tensor.matmul(
                            out=ps_r[:], lhsT=m1t_sb[:, c, kt, 0, 0:128],
                            rhs=dut_sb[:, kt, 0, ks], start=(kt == 0), stop=False)
                    for kt in range(4):
                        nc.tensor.matmul(
                            out=ps_r[:], lhsT=m1t_sb[:, c, kt, 1, 0:128],
                            rhs=dut_sb[:, kt, 2, ks], start=False, stop=(kt == 3))
                    for kt in range(4):
                        nc.tensor.matmul(
                            out=ps_i[:], lhsT=m1t_sb[:, c, kt, 0, 0:128],
                            rhs=dut_sb[:, kt, 1, ks], start=(kt == 0), stop=False)
                    for kt in range(4):
                        nc.tensor.matmul(
                            out=ps_i[:], lhsT=m1t_sb[:, c, kt, 1, 0:128],
                            rhs=dut_sb[:, kt, 0, ks], start=False, stop=(kt == 3))
                    nc.vector.tensor_copy(out=f_main[:, ks, c, 0], in_=ps_r[:])
                    nc.vector.tensor_copy(out=f_main[:, ks, c, 1], in_=ps_i[:])
            # tail rows k1 in [128, 134): padded lhsT so each (c,ri) group of
            # 6 tail rows lands on a 32-aligned output partition.
            tailpack2 = wpool.tile([128, 4, 2, 2, 2, 32], _BF16)  # [m2p, kt, ch, cl, ri, 32]
            nc.vector.memset(tailpack2[:], 0.0)
            for kt in range(4):
                for c in range(NCHAN):
                    nc.vector.tensor_copy(
                        out=tailpack2[:, kt, c // 2, c % 2, :, 0:6],
                        in_=m1t_sb[:, c, kt, :, 128:134])
            for half in range(2):
                ks = bass.ts(half, 512)
                for ch in range(2):   # c half: channels (2*ch, 2*ch+1)
                    o1 = psB.tile([128, 512], _F32, tag="psB")
                    o2 = psB.tile([128, 512], _F32, tag="psB")
                    for kt in range(4):
                        nc.tensor.matmul(
                            out=o1[:], lhsT=tailpack2[:, kt, ch, :, :, :],
                            rhs=dut_sb[:, kt, 0, ks], start=(kt == 0), stop=(kt == 3))
                    for kt in range(4):
                        nc.tensor.matmul(
                            out=o2[:], lhsT=tailpack2[:, kt, ch, :, :, :],
                            rhs=dut_sb[:, kt, 1, ks], start=(kt == 0), stop=(kt == 3))
                    # rows cl*64 + ri*32 + t:  o1 = M1_ri*Dur, o2 = M1_ri*Dui
                    o2sb = spool.tile([128, 512], _F32, tag="o2sb")
                    nc.scalar.copy(out=o2sb[:], in_=o2[:])
                    for cl in range(2):
                        c = 2 * ch + cl
                        # F_r tail = o1[ri=0] - o2[ri=1] ; F_i = o2[ri=0] + o1[ri=1]
                        r0 = cl * 64
                        r1 = cl * 64 + 32
                        nc.vector.tensor_tensor(
                            out=f_tail[:, ks, c, 0], in0=o1[r0:r0 + 6, :],
                            in1=o2sb[r1:r1 + 6, :], op=mybir.AluOpType.subtract)
                        nc.vector.tensor_tensor(
                            out=f_tail[:, ks, c, 1], in0=o1[r1:r1 + 6, :],
                            in1=o2sb[r0:r0 + 6, :], op=mybir.AluOpType.add)

            ctxB.__exit__(None, None, None)

            # ---- conv along v -> G_p, streamed to HBM gbuf ----
            # gbuf element addr = v0*ROWPTS*PAYLOAD + u*PAYLOAD + (c*2+ri)*P + p
            f_main_v = f_main[:].rearrange("v k c r -> v (k c r)")
            f_tail_v = f_tail[:].rearrange("v k c r -> v (k c r)")
            NCH = 16  # N-chunks of 512 over (k2, c, ri) = 8192
            ctxC = tc.tile_pool(name="psC", bufs=2, space="PSUM")
            psC = ctxC.__enter__()
            for w in range(NCH):
                ns = bass.ts(w, 512)
                # interleave all 8 p-planes into [v0, pt, cr, p] in SBUF,
                # then write one contiguous 16KB-per-row DMA
                g4k = spool.tile([128, 64, 8, P], _F32, tag="g4k")
                for p in range(P):
                    gps = psC.tile([128, 512], _F32, tag="psG")
                    nc.tensor.matmul(out=gps[:], lhsT=w1t_sb[:, p, :],
                                     rhs=f_main_v[:, ns], start=True, stop=False)
                    # halo rows [96,128): accumulate tail contribution in PSUM
                    nc.tensor.matmul(out=gps[:], lhsT=w2_sb[:, p, :],
                                     rhs=f_tail_v[:, ns], start=False, stop=True)
                    nc.scalar.copy(
                        out=g4k[:, :, :, p],
                        in_=gps[:].rearrange("v (pt cr) -> v pt cr", pt=64))
                dst = bass.AP(
                    gbuf[:].tensor,
                    (w * 64) * PAYLOAD,
                    [[ROWPTS * PAYLOAD, 128], [1, 64 * PAYLOAD]],
                )
                nc.sync.dma_start(out=dst, in_=g4k[:].rearrange("v a b c -> v (a b c)"))
            ctxC.__exit__(None, None, None)
            # wrap duplication: points [0,6) -> [1024, 1030)
            dup_src = bass.AP(gbuf[:].tensor, 0,
                              [[ROWPTS * PAYLOAD, 128], [1, 6 * PAYLOAD]])
            dup_dst = bass.AP(gbuf[:].tensor, GRID * PAYLOAD,
                              [[ROWPTS * PAYLOAD, 128], [1, 6 * PAYLOAD]])
            nc.sync.dma_start(out=dup_dst, in_=dup_src)

            # ---- gather + reduce ----
            out_sb = wpool.tile([128, NSUB, NBLK, 8], _F32)
            for s in range(NSUB):
                sub_rows = SUBROWS[s]
                idx_space = (sub_rows - 1) * ROWPTS + GRID
                src = bass.AP(gbuf[:].tensor, ROW0[s] * ROWPTS * PAYLOAD,
                              [[PAYLOAD, idx_space], [1, ELEM]])
                for h in range(NCHUNK):
                    xt = gpool.tile([128, CBLK, ELEM], _F32, tag="xt")
                    idx_ap = idx_sb[:, s, h * (CIDX // 16):(h + 1) * (CIDX // 16)]
                    nc.gpsimd.dma_gather(
                        xt[:], src, idx_ap, CIDX, CIDX, ELEM, elem_step=PAYLOAD,
                        single_packet=False)
                    cb = s * NBLK + h * CBLK     # frac column base
                    # fv/fu powers [128, CBLK, P]
                    fvp = spool.tile([128, CBLK, P], _F32, tag="fvp")
                    fup = spool.tile([128, CBLK, P], _F32, tag="fup")
                    for (pw, fcol) in ((fvp, 0), (fup, 1)):
                        nc.vector.memset(pw[:, :, 0], 1.0)
                        nc.vector.tensor_copy(
                            out=pw[:, :, 1],
                            in_=frac_sb[:, fcol, cb:cb + CBLK])
                        for k in range(2, P):
                            nc.vector.tensor_tensor(
                                out=pw[:, :, k], in0=pw[:, :, k - 1],
                                in1=frac_sb[:, fcol, cb:cb + CBLK],
                                op=mybir.AluOpType.mult)
                    # wu[128, CBLK, J] = sum_p A_wu[j,p] * fup^p  (CELL^2 folded)
                    wuw = spool.tile([128, CBLK, J, P], _F32, tag="wuw")
                    nc.vector.tensor_tensor(
                        out=wuw[:],
                        in0=fup[:].unsqueeze(2).broadcast_to([128, CBLK, J, P]),
                        in1=awu_sb[:].rearrange("q (j p) -> q j p", j=J)
                            .unsqueeze(1).broadcast_to([128, CBLK, J, P]),
                        op=mybir.AluOpType.mult)
                    wut = spool.tile([128, CBLK, J], _F32, tag="wut")
                    nc.vector.tensor_reduce(
                        out=wut[:], in_=wuw[:], axis=mybir.AxisListType.X,
                        op=mybir.AluOpType.add)
                    # expand fv powers across u: [128, CBLK*6, P]
                    fvx = spool.tile([128, CBLK, J, P], _F32, tag="fvx")
                    nc.vector.tensor_copy(
                        out=fvx[:],
                        in_=fvp[:].unsqueeze(2).broadcast_to([128, CBLK, J, P]))
                    # stage 1: multiply by fv powers, reduce p  (p innermost)
                    xv = xt[:].rearrange("q b (u cr p) -> q (b u) cr p", u=J, cr=8)
                    nc.vector.tensor_tensor(
                        out=xv,
                        in0=xv,
                        in1=fvx[:].rearrange("q b u p -> q (b u) p")
                            .unsqueeze(2).broadcast_to([128, CBLK * J, 8, P]),
                        op=mybir.AluOpType.mult)
                    # reduce innermost p; write Y as [b, cr, u] (u innermost)
                    y = gpool.tile([128, CBLK, 8, J], _F32, tag="y")
                    yw = bass.AP(y[:].tensor, y[:].offset,
                                 [y[:].ap[0], [8 * J, CBLK], [1, J], [J, 8]])
                    nc.vector.tensor_reduce(
                        out=yw,
                        in_=xt[:].rearrange("q b (u cr p) -> q b u cr p", u=J, cr=8),
                        axis=mybir.AxisListType.X, op=mybir.AluOpType.add)
                    # stage 2: multiply by wu (in place), reduce u
                    nc.vector.tensor_tensor(
                        out=y[:], in0=y[:],
                        in1=wut[:].unsqueeze(2).broadcast_to([128, CBLK, 8, J]),
                        op=mybir.AluOpType.mult)
                    nc.vector.tensor_reduce(
                        out=out_sb[:, s, h * CBLK:(h + 1) * CBLK, :],
                        in_=y[:], axis=mybir.AxisListType.X,
                        op=mybir.AluOpType.add)
            nc.sync.dma_start(
                out=out_dram[:],
                in_=out_sb[:].rearrange("q s b e -> q (s b e)"))
    if compile:
        nc.compile()
    return nc


_PROGRAM = None


def _program():
    global _PROGRAM
    if _PROGRAM is None:
        _PROGRAM = _build_program()
    return _PROGRAM


# ---------------- host sharding / unsharding ----------------
def _bin_visibilities(uu, vv):
    """Returns per-core host data + bookkeeping for unsharding."""
    gv = vv.astype(np.float64) * GSCALE
    gu = uu.astype(np.float64) * GSCALE
    bv = np.floor(gv)
    bu = np.floor(gu)
    fv = (gv - bv).astype(np.float32)
    fu = (gu - bu).astype(np.float32)
    bvi = (bv.astype(np.int64)) % GRID
    bui = (bu.astype(np.int64)) % GRID
    core = bvi // BAND
    vloc = bvi % BAND
    sub = np.searchsorted(np.array(ROW0), vloc, side="right") - 1
    ubase = (bui - 2) % GRID
    idx = (vloc - np.array(ROW0)[sub]) * ROWPTS + ubase

    NSUB = len(SUBROWS)
    per_core = []
    overflow = []
    for c in range(NCORES):
        idx_arr = np.zeros((NSUB, NSUBPAD), np.int16)
        fv_arr = np.zeros((NSUB, NSUBPAD), np.float32)
        fu_arr = np.zeros((NSUB, NSUBPAD), np.float32)
        slots = np.full((NSUB, NSUBPAD), -1, np.int64)
        for s in range(NSUB):
            sel = np.nonzero((core == c) & (sub == s))[0]
            if len(sel) > NSUBPAD:
                overflow.extend(sel[NSUBPAD:].tolist())
                sel = sel[:NSUBPAD]
            n = len(sel)
            idx_arr[s, :n] = idx[sel].astype(np.int16)
            fv_arr[s, :n] = fv[sel]
            fu_arr[s, :n] = fu[sel]
            slots[s, :n] = sel
        per_core.append((idx_arr, fv_arr, fu_arr, slots))
    return per_core, overflow, (fv, fu, bvi, bui)


def _host_fallback(cube, uu, vv, vis_ids):
    """Exact reference computation for overflow visibilities (rare)."""
    if not len(vis_ids):
        return None
    vis_ids = np.asarray(vis_ids, np.int64)
    shifted = np.fft.fftshift(cube.astype(np.float64), axes=(1, 2))
    n = (np.arange(NPIX) - NPIX // 2) / GRID
    sc = 1.0 / _kb_ft(n)
    img = shifted * (sc[:, None] * sc[None, :])
    pad = (GRID - NPIX) // 2
    img = np.pad(img, ((0, 0), (pad, pad), (pad, pad)))
    F = np.fft.fft2(np.fft.ifftshift(img, axes=(1, 2)))
    gv = vv[vis_ids].astype(np.float64) * GSCALE
    gu = uu[vis_ids].astype(np.float64) * GSCALE
    out = np.zeros((NCHAN, len(vis_ids)), np.complex128)
    for t, (gvt, gut) in enumerate(zip(gv, gu)):
        bv, bu = np.floor(gvt), np.floor(gut)
        ivs = (int(bv) + np.arange(J) - 2) % GRID
        ius = (int(bu) + np.arange(J) - 2) % GRID
        wv = _kb((gvt - bv) - (np.arange(J) - 2))
        wu = _kb((gut - bu) - (np.arange(J) - 2))
        blockF = F[:, ivs[:, None], ius[None, :]]
        out[:, t] = np.einsum("cij,i,j->c", blockF, wv, wu)
    return (CELL_SIZE ** 2) * out


def _wrap_idx(arr):
    """[NSUBPAD] -> [128, NSUBPAD//16] wrapped in 16 partitions, x8 groups."""
    w = arr.reshape(-1, 16).T            # [16, NSUBPAD//16]
    return np.tile(w, (8, 1)).astype(np.int16)


def _perm128(arr):
    """[NSUBPAD] -> [128, NBLK]: element i -> (i%128, i//128)."""
    return np.ascontiguousarray(arr.reshape(-1, 128).T)


def kernel(cube, uu, vv):
    cst = _consts()
    nc = _program()
    per_core, overflow, _ = _bin_visibilities(uu, vv)

    bf = mybir.dt.np(_BF16)
    shared = {
        "cube": np.ascontiguousarray(cube, np.float32),
        "dut_r": cst.DuT_r.astype(bf),
        "dut_i": cst.DuT_i.astype(bf),
        "dut_ni": cst.DuT_ni.astype(bf),
        "w1t": np.ascontiguousarray(cst.W1T).astype(bf),
        "w2": cst.W2.astype(bf),
        "awu": np.ascontiguousarray(cst.A_wu.reshape(-1), np.float32),
    }
    NSUB = len(SUBROWS)
    in_maps = []
    for c in range(NCORES):
        idx_arr, fv_arr, fu_arr, _slots = per_core[c]
        idx_w = np.stack([_wrap_idx(idx_arr[s]) for s in range(NSUB)])
        fvp = np.concatenate([_perm128(fv_arr[s]) for s in range(NSUB)], axis=1)
        fup = np.concatenate([_perm128(fu_arr[s]) for s in range(NSUB)], axis=1)
        in_maps.append({
            **shared,
            "dvt_r": cst.DvT_r[c].astype(bf),
            "dvt_i": cst.DvT_i[c].astype(bf),
            "idx": idx_w,
            "frac": np.stack([fvp, fup]),
        })

    res = run_bass_kernel_spmd(nc, in_maps, list(range(NCORES)))

    out = np.zeros((NCHAN, NVIS), np.complex64)
    for c in range(NCORES):
        o = res.results[c]["out"].reshape(128, NSUB, NBLK, NCHAN, 2)
        _idx, _fv, _fu, slots = per_core[c]
        for s in range(NSUB):
            sl = slots[s]
            valid = sl >= 0
            i = np.nonzero(valid)[0]
            if not len(i):
                continue
            vals = o[i % 128, s, i // 128, :, :]      # [n, c, ri]
            out[:, sl[i]] = (vals[:, :, 0] + 1j * vals[:, :, 1]).T
    if overflow:
        fb = _host_fallback(np.asarray(cube), np.asarray(uu), np.asarray(vv), overflow)
        out[:, np.asarray(overflow, np.int64)] = fb.astype(np.complex64)
    return out


# revision 25
# speedup vs baseline: 1.0464x; 1.0464x over previous
"""NuFFT forward (cube -> visibilities) on 8 Trainium2 NeuronCores.

Algorithm (validated in numpy, rel err ~3.7e-3 absmax vs fp32 reference):
  1. F = FFT2(pad(deapodize(fftshift(cube))))  computed as two DFT matmuls
     with fftshift/deapodize/pad/ifftshift folded into host-precomputed
     complex DFT matrices (bf16 inputs, fp32 PSUM accumulation).
     Each core computes only its 134-row v-band of the 1024x1024 grid.
  2. Kaiser-Bessel interpolation via "polynomial-in-v" trick:
     wv[m,i] = sum_p A[i,p] * fv^p  (degree-7 fit, max rel err 7e-6), so
       out[c,m] = sum_j wu[m,j] sum_p fv[m]^p G_p[c, bv[m], bu[m]+j-2]
     where G_p = conv_v(F, A[:,p]) is computed as a banded matmul on PE.
     G is stored in HBM as [v, u, (c,ri,p)] so each visibility needs ONE
     contiguous 1536B gather (6 u-points x 256B), done with dma_gather.
  3. On-chip reduce (DVE): contract p with fv-powers, then j with wu.

Sharding: vis binned by bv into 8 v-bands (one per core), then into 5
v-subbands per core to keep dma_gather's int16 index space < 32767.
Overflowing buckets (pathological inputs) fall back to exact host compute.
"""

import numpy as np

import concourse.bass as bass
import concourse.bacc as bacc
import concourse.mybir as mybir
import concourse.tile as tile
from concourse import library_config
from concourse.bass_utils import run_bass_kernel_spmd

# ---------------- problem constants (hardcoded per spec) ----------------
NPIX = 512
NCHAN = 4
NVIS = 131072
GRID = 1024
J = 6
ALPHA = 2.34 * J
CELL_SIZE = 0.005
ARCSEC = np.pi / (180.0 * 3600.0)
DL = CELL_SIZE * ARCSEC
GSCALE = 1000.0 * DL * GRID      # g = klambda * GSCALE (grid units)

NCORES = 8
P = 8                            # polynomial terms per KB tap
BAND = GRID // NCORES            # 128 v-rows per core
FROWS = BAND + J                 # F band rows incl halo (134)
SUBROWS = (26, 26, 26, 26, 24)   # v-subband split (int16 index space)
ROW0 = (0, 26, 52, 78, 104)
ROWPTS = GRID + 6                # u points per stored G row (wrap dup)
PAYLOAD = 2 * NCHAN * P          # f32 per u-point: (c,ri,p) = 64 -> 256B
ELEM = 6 * PAYLOAD               # gathered elem: 384 f32 = 1536B
NSUBPAD = 3584                   # padded vis per (core, subband)
NBLK = NSUBPAD // 128            # 28
NCHUNK = 2                       # gather calls per subband
CIDX = NSUBPAD // NCHUNK         # 1792 idxs per gather call
CBLK = CIDX // 128               # 14 blocks per chunk
GBUF_F32 = BAND * ROWPTS * PAYLOAD   # per-core G buffer (f32 elems)

_F32 = mybir.dt.float32
_BF16 = mybir.dt.bfloat16
_I16 = mybir.dt.int16


# ---------------- host-side constant precompute ----------------
def _kb(d):
    t = 1.0 - (2.0 * d / J) ** 2
    with np.errstate(invalid="ignore"):
        v = np.where(t > 0, np.i0(ALPHA * np.sqrt(np.maximum(t, 0.0))) / J, 0.0)
    return v


def _kb_ft(u):
    z = np.sqrt((np.pi * J * u) ** 2 - ALPHA ** 2 + 0j)
    return np.real(np.sin(z) / z)


def _poly_coefs():
    """A[i, p]: kb(f - (i-2)) ~= sum_p A[i,p] f^p on f in [0,1)."""
    f = np.linspace(0.0, 1.0, 4001)
    V = np.vander(f, P, increasing=True)
    A = np.stack([np.linalg.lstsq(V, _kb(f - (i - 2)), rcond=None)[0]
                  for i in range(J)])
    return A  # float64 [J, P]


def _dft_matrices():
    """Effective DFT matrices absorbing fftshift, de-apodization, padding
    and ifftshift.  D[k, m] applied to raw cube index m gives spectrum
    row/col k of the oversampled grid."""
    n = (np.arange(NPIX) - NPIX // 2) / GRID
    sc = 1.0 / _kb_ft(n)                       # de-apod per img index n1
    m = np.arange(NPIX)
    n_of_m = (m + 256) % NPIX                  # img row for cube row m
    phi = (n_of_m - 256) % GRID                # padded+ifftshifted position
    k = np.arange(GRID)
    D = sc[n_of_m][None, :] * np.exp(-2j * np.pi * k[:, None] * phi[None, :] / GRID)
    return D  # complex128 [GRID, NPIX]


class _Consts:
    def __init__(self):
        A = _poly_coefs()
        self.A = A
        D = _dft_matrices()
        bf = lambda x: x.astype(np.float32).astype(np.float32)  # keep f32; cast on pack
        self.D = D
        # DuT variants [NPIX, GRID] bf16: real, imag, -imag
        DuT = D.T.copy()
        self.DuT_r = DuT.real.astype(np.float32)
        self.DuT_i = DuT.imag.astype(np.float32)
        self.DuT_ni = (-DuT.imag).astype(np.float32)
        # Per-core DvT bands [NPIX, FROWS]
        self.DvT_r = []
        self.DvT_i = []
        for c in range(NCORES):
            rows = (c * BAND - 2 + np.arange(FROWS)) % GRID
            Db = D[rows, :].T.copy()     # [NPIX, FROWS]
            self.DvT_r.append(Db.real.astype(np.float32))
            self.DvT_i.append(Db.imag.astype(np.float32))
        # Conv matrices.  G_p[v0] = sum_i A[i,p] F_local[v0+i],
        # F_local row r = global band row (c*BAND - 2 + r).
        W1 = np.zeros((P, 128, 128), np.float32)   # [p, r, v0] for r<128
        for p in range(P):
            for v0 in range(128):
                for i in range(J):
                    r = v0 + i
                    if r < 128:
                        W1[p, r, v0] = A[i, p]
        self.W1T = W1                               # lhsT layout [K=r, M=v0]
        # halo: r in [128, 134) -> v0 in [123,128): lhsT [6, P*5]
        # halo lhsT per p: [J, 32] columns = (v0 - 96), nonzero v0 >= 123;
        # accumulated into the conv PSUM rows [96,128) via a 2nd matmul
        W2 = np.zeros((P, J, 128), np.float32)
        for p in range(P):
            for v0 in range(123, 128):
                for i in range(J):
                    r = v0 + i
                    if r >= 128:
                        W2[p, r - 128, v0] = A[i, p]
        self.W2 = W2
        # A replicated for on-device wu computation, scaled by CELL^2
        self.A_wu = (A * (CELL_SIZE ** 2)).astype(np.float32)  # [J, P]


_CONSTS = None


def _consts():
    global _CONSTS
    if _CONSTS is None:
        _CONSTS = _Consts()
    return _CONSTS


def _to_bf16(x):
    return np.asarray(x, dtype=np.float32).astype(mybir.dt.np(_BF16))


# ---------------- device program ----------------
def _build_program(compile=True):
    cst = _consts()
    nc = bacc.Bacc("TRN2", target_bir_lowering=False, debug=False)

    cube_in = nc.declare_dram_parameter("cube", [NCHAN, NPIX, NPIX], _F32, isOutput=False)
    dvt_r = nc.declare_dram_parameter("dvt_r", [NPIX, FROWS], _BF16, isOutput=False)
    dvt_i = nc.declare_dram_parameter("dvt_i", [NPIX, FROWS], _BF16, isOutput=False)
    dut_r = nc.declare_dram_parameter("dut_r", [NPIX, GRID], _BF16, isOutput=False)
    dut_i = nc.declare_dram_parameter("dut_i", [NPIX, GRID], _BF16, isOutput=False)
    dut_ni = nc.declare_dram_parameter("dut_ni", [NPIX, GRID], _BF16, isOutput=False)
    w1t_in = nc.declare_dram_parameter("w1t", [P, 128, 128], _BF16, isOutput=False)
    w2_in = nc.declare_dram_parameter("w2", [P, J, 128], _BF16, isOutput=False)
    awu_in = nc.declare_dram_parameter("awu", [J * P], _F32, isOutput=False)
    idx_in = nc.declare_dram_parameter("idx", [len(SUBROWS), 128, NSUBPAD // 16], _I16, isOutput=False)
    frac_in = nc.declare_dram_parameter("frac", [2, 128, len(SUBROWS) * NBLK], _F32, isOutput=False)
    out_dram = nc.declare_dram_parameter("out", [128, len(SUBROWS) * NBLK * 8], _F32, isOutput=True)

    gbuf = nc.dram_tensor("gbuf", [GBUF_F32], _F32)

    NSUB = len(SUBROWS)

    with tile.TileContext(nc) as tc:
        with (
            tc.tile_pool(name="const", bufs=1) as cpool,
            tc.tile_pool(name="work", bufs=1) as wpool,
            tc.tile_pool(name="gather", bufs=2) as gpool,
            tc.tile_pool(name="stage", bufs=2) as spool,
        ):
            nc.gpsimd.load_library(library_config.mlp)

            # ---- constant loads ----
            cube_sb = cpool.tile([128, NCHAN, 4, NPIX], _BF16)   # [m1p, c, m1t, m2]
            for t in range(4):
                nc.gpsimd.dma_start(
                    out=cube_sb[:, :, t, :],
                    in_=cube_in[:, bass.ts(t, 128), :].transpose([1, 0, 2]),
                )
            dvt_sb = cpool.tile([128, 4, 2, FROWS], _BF16)       # [m1p, m1t, ri, k1]
            for t in range(4):
                nc.sync.dma_start(out=dvt_sb[:, t, 0, :], in_=dvt_r[bass.ts(t, 128), :])
                nc.sync.dma_start(out=dvt_sb[:, t, 1, :], in_=dvt_i[bass.ts(t, 128), :])
            dut_sb = cpool.tile([128, 4, 3, GRID], _BF16)        # [m2p, m2t, var, k2]
            for t in range(4):
                nc.sync.dma_start(out=dut_sb[:, t, 0, :], in_=dut_r[bass.ts(t, 128), :])
                nc.sync.dma_start(out=dut_sb[:, t, 1, :], in_=dut_i[bass.ts(t, 128), :])
                nc.sync.dma_start(out=dut_sb[:, t, 2, :], in_=dut_ni[bass.ts(t, 128), :])
            w1t_sb = cpool.tile([128, P, 128], _BF16)
            nc.sync.dma_start(out=w1t_sb[:], in_=w1t_in[:].transpose([1, 0, 2]))
            w2_sb = cpool.tile([J, P, 128], _BF16)
            nc.sync.dma_start(out=w2_sb[:], in_=w2_in[:].transpose([1, 0, 2]))
            awu_sb = cpool.tile([128, J * P], _F32)
            nc.sync.dma_start(out=awu_sb[:], in_=awu_in[None, :].broadcast_to([128, J * P]))
            idx_sb = cpool.tile([128, NSUB, NSUBPAD // 16], _I16)
            nc.sync.dma_start(out=idx_sb[:], in_=idx_in[:].transpose([1, 0, 2]))
            frac_sb = cpool.tile([128, 2, NSUB * NBLK], _F32)
            nc.sync.dma_start(out=frac_sb[:], in_=frac_in[:].transpose([1, 0, 2]))

            # ---- stage A: M1T[m2, k1] = (cube^T @ DvT) per (c, ri) ----
            m1t_sb = wpool.tile([128, NCHAN, 4, 2, FROWS], _BF16)  # [m2p, c, m2blk, ri, k1]
            psA = ctxA = tc.tile_pool(name="psA", bufs=4, space="PSUM")
            psA = ctxA.__enter__()
            for c in range(NCHAN):
                for blk in range(4):
                    for ri in range(2):
                        ps = psA.tile([128, FROWS], _F32, tag="psA")
                        for kt in range(4):
                            nc.tensor.matmul(
                                out=ps[:],
                                lhsT=cube_sb[:, c, kt, bass.ts(blk, 128)],
                                rhs=dvt_sb[:, kt, ri, :],
                                start=(kt == 0),
                                stop=(kt == 3),
                            )
                        nc.vector.tensor_copy(out=m1t_sb[:, c, blk, ri, :], in_=ps[:])

            ctxA.__exit__(None, None, None)

            # ---- stage B: F[k1, k2] = M1 @ Du^T  (k1 in band, 134 rows) ----
            # F_sb layout: [v-part, (k2, c, ri)] bf16; main 128 rows + 6-row tail
            f_main = wpool.tile([128, GRID, NCHAN, 2], _BF16)
            f_tail = wpool.tile([6, GRID, NCHAN, 2], _BF16)
            ctxB = tc.tile_pool(name="psB", bufs=4, space="PSUM")
            psB = ctxB.__enter__()
            for c in range(NCHAN):
                for half in range(2):
                    ks = bass.ts(half, 512)
                    ps_r = psB.tile([128, 512], _F32, tag="psB")
                    ps_i = psB.tile([128, 512], _F32, tag="psB")
                    # F_r = M1r*Dur + M1i*(-Dui);  F_i = M1r*Dui + M1i*Dur
                    for kt in range(4):
                        nc.tensor.matmul(
                            out=ps_r[:], lhsT=m1t_sb[:, c, kt, 0, 0:128],
                            rhs=dut_sb[:, kt, 0, ks], start=(kt == 0), stop=False)
                    for kt in range(4):
                        nc.tensor.matmul(
                            out=ps_r[:], lhsT=m1t_sb[:, c, kt, 1, 0:128],
                            rhs=dut_sb[:, kt, 2, ks], start=False, stop=(kt == 3))
                    for kt in range(4):
                        nc.tensor.matmul(
                            out=ps_i[:], lhsT=m1t_sb[:, c, kt, 0, 0:128],
                            rhs=dut_sb[:, kt, 1, ks], start=(kt == 0), stop=False)
                    for kt in range(4):
                        nc.tensor.matmul(
                            out=ps_i[:], lhsT=m1t_sb[:, c, kt, 1, 0:128],
                            rhs=dut_sb[:, kt, 0, ks], start=False, stop=(kt == 3))
                    nc.vector.tensor_copy(out=f_main[:, ks, c, 0], in_=ps_r[:])
                    nc.vector.tensor_copy(out=f_main[:, ks, c, 1], in_=ps_i[:])
            # tail rows k1 in [128, 134): padded lhsT so each (c,ri) group of
            # 6 tail rows lands on a 32-aligned output partition.
            tailpack2 = wpool.tile([128, 4, 2, 2, 2, 32], _BF16)  # [m2p, kt, ch, cl, ri, 32]
            nc.vector.memset(tailpack2[:], 0.0)
            for kt in range(4):
                for c in range(NCHAN):
                    nc.vector.tensor_copy(
                        out=tailpack2[:, kt, c // 2, c % 2, :, 0:6],
                        in_=m1t_sb[:, c, kt, :, 128:134])
            for half in range(2):
                ks = bass.ts(half, 512)
                for ch in range(2):   # c half: channels (2*ch, 2*ch+1)
                    o1 = psB.tile([128, 512], _F32, tag="psB")
                    o2 = psB.tile([128, 512], _F32, tag="psB")
                    for kt in range(4):
                        nc.tensor.matmul(
                            out=o1[:], lhsT=tailpack2[:, kt, ch, :, :, :],
                            rhs=dut_sb[:, kt, 0, ks], start=(kt == 0), stop=(kt == 3))
                    for kt in range(4):
                        nc.tensor.matmul(
                            out=o2[:], lhsT=tailpack2[:, kt, ch, :, :, :],
                            rhs=dut_sb[:, kt, 1, ks], start=(kt == 0), stop=(kt == 3))
                    # rows cl*64 + ri*32 + t:  o1 = M1_ri*Dur, o2 = M1_ri*Dui
                    o2sb = spool.tile([128, 512], _F32, tag="o2sb")
                    nc.scalar.copy(out=o2sb[:], in_=o2[:])
                    for cl in range(2):
                        c = 2 * ch + cl
                        # F_r tail = o1[ri=0] - o2[ri=1] ; F_i = o2[ri=0] + o1[ri=1]
                        r0 = cl * 64
                        r1 = cl * 64 + 32
                        nc.vector.tensor_tensor(
                            out=f_tail[:, ks, c, 0], in0=o1[r0:r0 + 6, :],
                            in1=o2sb[r1:r1 + 6, :], op=mybir.AluOpType.subtract)
                        nc.vector.tensor_tensor(
                            out=f_tail[:, ks, c, 1], in0=o1[r1:r1 + 6, :],
                            in1=o2sb[r0:r0 + 6, :], op=mybir.AluOpType.add)

            ctxB.__exit__(None, None, None)

            # ---- conv along v -> G_p, streamed to HBM gbuf ----
            # gbuf element addr = v0*ROWPTS*PAYLOAD + u*PAYLOAD + (c*2+ri)*P + p
            f_main_v = f_main[:].rearrange("v k c r -> v (k c r)")
            f_tail_v = f_tail[:].rearrange("v k c r -> v (k c r)")
            NCH = 16  # N-chunks of 512 over (k2, c, ri) = 8192
            ctxC = tc.tile_pool(name="psC", bufs=2, space="PSUM")
            psC = ctxC.__enter__()
            for w in range(NCH):
                ns = bass.ts(w, 512)
                # interleave all 8 p-planes into [v0, pt, cr, p] in SBUF,
                # then write one contiguous 16KB-per-row DMA
                g4k = spool.tile([128, 64, 8, P], _F32, tag="g4k")
                for p in range(P):
                    gps = psC.tile([128, 512], _F32, tag="psG")
                    nc.tensor.matmul(out=gps[:], lhsT=w1t_sb[:, p, :],
                                     rhs=f_main_v[:, ns], start=True, stop=False)
                    # halo rows [96,128): accumulate tail contribution in PSUM
                    nc.tensor.matmul(out=gps[:], lhsT=w2_sb[:, p, :],
                                     rhs=f_tail_v[:, ns], start=False, stop=True)
                    nc.scalar.copy(
                        out=g4k[:, :, :, p],
                        in_=gps[:].rearrange("v (pt cr) -> v pt cr", pt=64))
                dst = bass.AP(
                    gbuf[:].tensor,
                    (w * 64) * PAYLOAD,
                    [[ROWPTS * PAYLOAD, 128], [1, 64 * PAYLOAD]],
                )
                nc.sync.dma_start(out=dst, in_=g4k[:].rearrange("v a b c -> v (a b c)"))
            ctxC.__exit__(None, None, None)
            # wrap duplication: points [0,6) -> [1024, 1030)
            dup_src = bass.AP(gbuf[:].tensor, 0,
                              [[ROWPTS * PAYLOAD, 128], [1, 6 * PAYLOAD]])
            dup_dst = bass.AP(gbuf[:].tensor, GRID * PAYLOAD,
                              [[ROWPTS * PAYLOAD, 128], [1, 6 * PAYLOAD]])
            nc.sync.dma_start(out=dup_dst, in_=dup_src)

            # ---- gather + reduce ----
            out_sb = wpool.tile([128, NSUB, NBLK, 8], _F32)
            for s in range(NSUB):
                sub_rows = SUBROWS[s]
                idx_space = (sub_rows - 1) * ROWPTS + GRID
                src = bass.AP(gbuf[:].tensor, ROW0[s] * ROWPTS * PAYLOAD,
                              [[PAYLOAD, idx_space], [1, ELEM]])
                for h in range(NCHUNK):
                    xt = gpool.tile([128, CBLK, ELEM], _F32, tag="xt")
                    idx_ap = idx_sb[:, s, h * (CIDX // 16):(h + 1) * (CIDX // 16)]
                    nc.gpsimd.dma_gather(
                        xt[:], src, idx_ap, CIDX, CIDX, ELEM, elem_step=PAYLOAD,
                        single_packet=False)
                    cb = s * NBLK + h * CBLK     # frac column base
                    # fv/fu powers [128, CBLK, P]
                    fvp = spool.tile([128, CBLK, P], _F32, tag="fvp")
                    fup = spool.tile([128, CBLK, P], _F32, tag="fup")
                    for (pw, fcol) in ((fvp, 0), (fup, 1)):
                        nc.vector.memset(pw[:, :, 0], 1.0)
                        nc.vector.tensor_copy(
                            out=pw[:, :, 1],
                            in_=frac_sb[:, fcol, cb:cb + CBLK])
                        for k in range(2, P):
                            nc.vector.tensor_tensor(
                                out=pw[:, :, k], in0=pw[:, :, k - 1],
                                in1=frac_sb[:, fcol, cb:cb + CBLK],
                                op=mybir.AluOpType.mult)
                    # wu[128, CBLK, J] = sum_p A_wu[j,p] * fup^p  (CELL^2 folded)
                    wuw = spool.tile([128, CBLK, J, P], _F32, tag="wuw")
                    nc.vector.tensor_tensor(
                        out=wuw[:],
                        in0=fup[:].unsqueeze(2).broadcast_to([128, CBLK, J, P]),
                        in1=awu_sb[:].rearrange("q (j p) -> q j p", j=J)
                            .unsqueeze(1).broadcast_to([128, CBLK, J, P]),
                        op=mybir.AluOpType.mult)
                    wut = spool.tile([128, CBLK, J], _F32, tag="wut")
                    nc.vector.tensor_reduce(
                        out=wut[:], in_=wuw[:], axis=mybir.AxisListType.X,
                        op=mybir.AluOpType.add)
                    # expand fv powers across u: [128, CBLK*6, P]
                    fvx = spool.tile([128, CBLK, J, P], _F32, tag="fvx")
                    nc.vector.tensor_copy(
                        out=fvx[:],
                        in_=fvp[:].unsqueeze(2).broadcast_to([128, CBLK, J, P]))
                    # stage 1: multiply by fv powers, reduce p  (p innermost).
                    # Halves run on DVE and GPSIMD in parallel to shorten the
                    # per-chunk gather->mul->reduce chain.
                    xv = xt[:].rearrange("q b (u cr p) -> q (b u) cr p", u=J, cr=8)
                    fvb = fvx[:].rearrange("q b u p -> q (b u) p")                         .unsqueeze(2).broadcast_to([128, CBLK * J, 8, P])
                    HB = CBLK * J // 2
                    nc.vector.tensor_tensor(
                        out=xv[:, 0:HB], in0=xv[:, 0:HB], in1=fvb[:, 0:HB],
                        op=mybir.AluOpType.mult)
                    nc.gpsimd.tensor_tensor(
                        out=xv[:, HB:], in0=xv[:, HB:], in1=fvb[:, HB:],
                        op=mybir.AluOpType.mult)
                    # reduce innermost p; write Y as [b, cr, u] (u innermost)
                    y = gpool.tile([128, CBLK, 8, J], _F32, tag="y")
                    yw = bass.AP(y[:].tensor, y[:].offset,
                                 [y[:].ap[0], [8 * J, CBLK], [1, J], [J, 8]])
                    nc.vector.tensor_reduce(
                        out=yw,
                        in_=xt[:].rearrange("q b (u cr p) -> q b u cr p", u=J, cr=8),
                        axis=mybir.AxisListType.X, op=mybir.AluOpType.add)
                    # stage 2: multiply by wu (in place), reduce u
                    nc.vector.tensor_tensor(
                        out=y[:], in0=y[:],
                        in1=wut[:].unsqueeze(2).broadcast_to([128, CBLK, 8, J]),
                        op=mybir.AluOpType.mult)
                    nc.vector.tensor_reduce(
                        out=out_sb[:, s, h * CBLK:(h + 1) * CBLK, :],
                        in_=y[:], axis=mybir.AxisListType.X,
                        op=mybir.AluOpType.add)
            nc.sync.dma_start(
                out=out_dram[:],
                in_=out_sb[:].rearrange("q s b e -> q (s b e)"))
    if compile:
        nc.compile()
    return nc


_PROGRAM = None


def _program():
    global _PROGRAM
    if _PROGRAM is None:
        _PROGRAM = _build_program()
    return _PROGRAM


# ---------------- host sharding / unsharding ----------------
def _bin_visibilities(uu, vv):
    """Returns per-core host data + bookkeeping for unsharding."""
    gv = vv.astype(np.float64) * GSCALE
    gu = uu.astype(np.float64) * GSCALE
    bv = np.floor(gv)
    bu = np.floor(gu)
    fv = (gv - bv).astype(np.float32)
    fu = (gu - bu).astype(np.float32)
    bvi = (bv.astype(np.int64)) % GRID
    bui = (bu.astype(np.int64)) % GRID
    core = bvi // BAND
    vloc = bvi % BAND
    sub = np.searchsorted(np.array(ROW0), vloc, side="right") - 1
    ubase = (bui - 2) % GRID
    idx = (vloc - np.array(ROW0)[sub]) * ROWPTS + ubase

    NSUB = len(SUBROWS)
    per_core = []
    overflow = []
    for c in range(NCORES):
        idx_arr = np.zeros((NSUB, NSUBPAD), np.int16)
        fv_arr = np.zeros((NSUB, NSUBPAD), np.float32)
        fu_arr = np.zeros((NSUB, NSUBPAD), np.float32)
        slots = np.full((NSUB, NSUBPAD), -1, np.int64)
        for s in range(NSUB):
            sel = np.nonzero((core == c) & (sub == s))[0]
            if len(sel) > NSUBPAD:
                overflow.extend(sel[NSUBPAD:].tolist())
                sel = sel[:NSUBPAD]
            n = len(sel)
            idx_arr[s, :n] = idx[sel].astype(np.int16)
            fv_arr[s, :n] = fv[sel]
            fu_arr[s, :n] = fu[sel]
            slots[s, :n] = sel
        per_core.append((idx_arr, fv_arr, fu_arr, slots))
    return per_core, overflow, (fv, fu, bvi, bui)


def _host_fallback(cube, uu, vv, vis_ids):
    """Exact reference computation for overflow visibilities (rare)."""
    if not len(vis_ids):
        return None
    vis_ids = np.asarray(vis_ids, np.int64)
    shifted = np.fft.fftshift(cube.astype(np.float64), axes=(1, 2))
    n = (np.arange(NPIX) - NPIX // 2) / GRID
    sc = 1.0 / _kb_ft(n)
    img = shifted * (sc[:, None] * sc[None, :])
    pad = (GRID - NPIX) // 2
    img = np.pad(img, ((0, 0), (pad, pad), (pad, pad)))
    F = np.fft.fft2(np.fft.ifftshift(img, axes=(1, 2)))
    gv = vv[vis_ids].astype(np.float64) * GSCALE
    gu = uu[vis_ids].astype(np.float64) * GSCALE
    out = np.zeros((NCHAN, len(vis_ids)), np.complex128)
    for t, (gvt, gut) in enumerate(zip(gv, gu)):
        bv, bu = np.floor(gvt), np.floor(gut)
        ivs = (int(bv) + np.arange(J) - 2) % GRID
        ius = (int(bu) + np.arange(J) - 2) % GRID
        wv = _kb((gvt - bv) - (np.arange(J) - 2))
        wu = _kb((gut - bu) - (np.arange(J) - 2))
        blockF = F[:, ivs[:, None], ius[None, :]]
        out[:, t] = np.einsum("cij,i,j->c", blockF, wv, wu)
    return (CELL_SIZE ** 2) * out


def _wrap_idx(arr):
    """[NSUBPAD] -> [128, NSUBPAD//16] wrapped in 16 partitions, x8 groups."""
    w = arr.reshape(-1, 16).T            # [16, NSUBPAD//16]
    return np.tile(w, (8, 1)).astype(np.int16)


def _perm128(arr):
    """[NSUBPAD] -> [128, NBLK]: element i -> (i%128, i//128)."""
    return np.ascontiguousarray(arr.reshape(-1, 128).T)


def kernel(cube, uu, vv):
    cst = _consts()
    nc = _program()
    per_core, overflow, _ = _bin_visibilities(uu, vv)

    bf = mybir.dt.np(_BF16)
    shared = {
        "cube": np.ascontiguousarray(cube, np.float32),
        "dut_r": cst.DuT_r.astype(bf),
        "dut_i": cst.DuT_i.astype(bf),
        "dut_ni": cst.DuT_ni.astype(bf),
        "w1t": np.ascontiguousarray(cst.W1T).astype(bf),
        "w2": cst.W2.astype(bf),
        "awu": np.ascontiguousarray(cst.A_wu.reshape(-1), np.float32),
    }
    NSUB = len(SUBROWS)
    in_maps = []
    for c in range(NCORES):
        idx_arr, fv_arr, fu_arr, _slots = per_core[c]
        idx_w = np.stack([_wrap_idx(idx_arr[s]) for s in range(NSUB)])
        fvp = np.concatenate([_perm128(fv_arr[s]) for s in range(NSUB)], axis=1)
        fup = np.concatenate([_perm128(fu_arr[s]) for s in range(NSUB)], axis=1)
        in_maps.append({
            **shared,
            "dvt_r": cst.DvT_r[c].astype(bf),
            "dvt_i": cst.DvT_i[c].astype(bf),
            "idx": idx_w,
            "frac": np.stack([fvp, fup]),
        })

    res = run_bass_kernel_spmd(nc, in_maps, list(range(NCORES)))

    out = np.zeros((NCHAN, NVIS), np.complex64)
    for c in range(NCORES):
        o = res.results[c]["out"].reshape(128, NSUB, NBLK, NCHAN, 2)
        _idx, _fv, _fu, slots = per_core[c]
        for s in range(NSUB):
            sl = slots[s]
            valid = sl >= 0
            i = np.nonzero(valid)[0]
            if not len(i):
                continue
            vals = o[i % 128, s, i // 128, :, :]      # [n, c, ri]
            out[:, sl[i]] = (vals[:, :, 0] + 1j * vals[:, :, 1]).T
    if overflow:
        fb = _host_fallback(np.asarray(cube), np.asarray(uu), np.asarray(vv), overflow)
        out[:, np.asarray(overflow, np.int64)] = fb.astype(np.complex64)
    return out
